# revision 1
# baseline (speedup 1.0000x reference)
"""Trainium2 Bass kernel for relative-position attention (nn_Attention).

Reference computation (B=16, C=128, H=W=32, HEADS=4, d=32, N=1024):
    qkv  = W_qkv @ x                          (1x1 conv, per-pixel matmul)
    S    = scale * (q^T k + q^T r)            where r = rw + rh  (broadcast)
         = scale * q^T (k + r)                <- position term folds into k
    P    = softmax(S, axis=-1)
    out  = P @ v^T

Sharding: data-parallel over batch, 2 batches per core on 8 cores.
W_qkv / rw / rh replicated. No collectives.

Per-core kernel layout (all on-chip, nothing round-trips to DRAM):
  - qkv projection: wT [128c, 384o] stationary, x_b [128c, 1024] moving.
  - S^T per (head, j-chunk): lhsT = (k+r)_h [32d, 128j], rhs = q_h [32d, 1024i]
    -> PSUM [128j, 1024i]; exp via ScalarE (no max-subtraction needed:
    logits are O(10), far below fp32 exp overflow).
  - O  per head: lhsT = [v^T | 1] [128j, 33], rhs = E [128j, 1024i],
    accumulated over 8 j-chunks -> PSUM [33, 1024]; row 32 is the softmax
    denominator Z (ones column trick shares the matmul stream).
  - normalize (entirely off the PE): Z row -> SBUF, DMA-repartitioned to
    [32,32] so the DVE reciprocal runs on 32 lanes instead of 1, bounced
    through a DRAM row and read back with a step-0 partition-broadcast AP,
    then out = O * (1/Z) on VectorE.

The previous head's O matmuls are interleaved between the current head's S
chunks so the PE (the bottleneck engine, saturated ~96-100%) streams
continuously instead of pacing itself to ScalarE's exp drain rate.

Matmuls run in float32r by default (reduced-mantissa fp32; measured 3.7e-4
output rel err vs 4e-3 for bf16, at ~15% lower speed). fp32r operands must
be produced rounded, so every tile feeding a matmul is written as float32r
by the producing engine op; q/k+r/E/vt get that for free from the copies
they already need, x and W each pay one extra rounding copy.

q and k+r are stored as head-pair tiles of 64 partitions because PE matmul
operands must base at partition 0/32/64 (quadrant 3 is unusable).
"""

import numpy as np

B, C, H, W = 16, 128, 32, 32
HEADS = 4
D = C // HEADS          # 32
N = H * W               # 1024
SCALE = float(D) ** -0.5
NCORES = 8
BPC = B // NCORES       # batches per core

# matmul input dtypes per stage; "f32r" = reduced-mantissa fp32 (measured
# ~3.7e-4 output rel err, within ~15% of bf16 speed on this part since the
# PE streams at 1.2 GHz either way), "bf16" = fastest but ~4e-3 rel err,
# "f32" = exact but 4 cycles/row. QKV_DTYPE covers the projection, S_DTYPE
# the q/(k+r) score matmul, O_DTYPE the E/v attention-apply matmul.
import os as _os
QKV_DTYPE = _os.environ.get("KQKV_DT", "f32r")
S_DTYPE = _os.environ.get("KS_DT", "f32r")
O_DTYPE = _os.environ.get("KO_DT", "f32r")


def _build_kernel(nc, tc, tile, mybir, x_ap, wT_ap, rw_ap, rh_ap, out_ap):
    import concourse.bass as bass
    from concourse.masks import make_identity

    f32 = mybir.dt.float32
    DT = {"f32r": mybir.dt.float32r, "f32": mybir.dt.float32,
          "bf16": mybir.dt.bfloat16}
    mdt = DT[QKV_DTYPE]
    sdt = DT[S_DTYPE]
    odt = DT[O_DTYPE]

    const = tc.alloc_tile_pool(name="const", bufs=1)
    sb = tc.alloc_tile_pool(name="sb", bufs=2)
    epool = tc.alloc_tile_pool(name="epool", bufs=20)
    vt1pool = tc.alloc_tile_pool(name="vt1pool", bufs=16)
    psmm = tc.alloc_tile_pool(name="psmm", bufs=2, space="PSUM")
    psacc = tc.alloc_tile_pool(name="psacc", bufs=2, space="PSUM")
    dscratch = tc.alloc_tile_pool(name="dscratch", bufs=4, space="DRAM")

    # --- constants / replicated inputs ---
    identity = const.tile([128, 128], f32)
    make_identity(nc, identity[:])
    ones_f = const.tile([128, 32], f32)
    nc.vector.memset(ones_f[:], 1.0)
    # prefetch batch 0's x before the (smaller) weight DMAs so the first
    # qkv matmul's moving operand is ready sooner
    x0_s = sb.tile([128, N], f32, tag="x", name="x0_s")
    for nf in range(2):
        nc.sync.dma_start(out=x0_s[:, nf * 512:(nf + 1) * 512],
                          in_=x_ap[0, :, nf * 512:(nf + 1) * 512])
    w_s = const.tile([128, 3 * C], f32)
    nc.sync.dma_start(out=w_s[:], in_=wT_ap[:])
    rw_s = const.tile([128, W], f32)
    nc.sync.dma_start(out=rw_s[:], in_=rw_ap[:])
    rh_s = const.tile([128, H], f32)
    nc.sync.dma_start(out=rh_s[:], in_=rh_ap[:])

    if mdt != f32:
        w_r = const.tile([128, 3 * C], mdt)
        nc.vector.tensor_copy(out=w_r[:], in_=w_s[:])
    else:
        w_r = w_s

    # r[p, y*W + x] = rw[p, x] + rh[p, y] in one DVE op via step-0 free dims
    r_s = const.tile([128, N], f32)
    rw_b = bass.AP(tensor=rw_s.tensor, offset=rw_s.offset,
                   ap=[list(rw_s.ap[0]), [0, H], list(rw_s.ap[1])])
    rh_b = bass.AP(tensor=rh_s.tensor, offset=rh_s.offset,
                   ap=[list(rh_s.ap[0]), list(rh_s.ap[1]), [0, W]])
    nc.vector.tensor_add(
        out=r_s[:].rearrange("p (y x) -> p y x", y=H), in0=rh_b, in1=rw_b
    )

    # Software pipelining: the previous head's 16 O matmuls are interleaved
    # between the current head's S chunks (2 O chunks after each S chunk), so
    # the PE streams continuously instead of pacing itself to ScalarE's exp
    # drain rate (2 PSUM score slots). A continuously-busy PE also keeps the
    # HAM clock-gate warm (2.4 GHz vs 1.2 GHz). The previous head's
    # normalize runs entirely off the PE (DVE reciprocal + DMA broadcast)
    # and is spliced in one head later still.
    prev_o = []       # pending O-matmul thunks for the previous head
    pending = []      # pending normalize tails

    def emit_pending():
        while pending:
            pending.pop(0)()

    def finish_head(h, b, ps_o, out_s):
        """Emit after head h's O matmuls: off-PE normalize chain.
        Z row -> SBUF, DMA-repartition [1,1024]->[32,32] so the reciprocal
        runs on 32 DVE lanes instead of 1, DMA to a DRAM bounce row, read it
        back partition-broadcast, then out = O * R."""
        z_c = sb.tile([1, N], f32, tag="zc", name=f"zc{h}")
        nc.scalar.copy(out=z_c[:], in_=ps_o[D:D + 1, :])
        z32 = sb.tile([D, H], f32, tag="z32", name=f"z32_{h}")
        nc.sync.dma_start(out=z32[:], in_=z_c[:])
        rz32 = sb.tile([D, H], f32, tag="rz32", name=f"rz32_{h}")
        nc.vector.reciprocal(out=rz32[:], in_=z32[:])
        r_d = dscratch.tile([1, N], f32, tag="rd", name=f"rd{h}")
        nc.sync.dma_start(out=r_d[:], in_=rz32[:])
        # issue the broadcast read eagerly so its ~2us completion-semaphore
        # latency overlaps other work instead of stalling the deferred mul
        rb = sb.tile([D, N], f32, tag="rb")
        nc.sync.dma_start(out=rb[:], in_=r_d[0, :].partition_broadcast(D))

        def norm_tail():
            nc.vector.tensor_mul(
                out=out_s[h * D:(h + 1) * D, :], in0=ps_o[0:D, :], in1=rb[:]
            )
            # per-head output DMA so the kernel tail only waits on the last
            # head's 128KB slice, not the whole batch
            nc.sync.dma_start(
                out=out_ap[b, h * D:(h + 1) * D, :],
                in_=out_s[h * D:(h + 1) * D, :],
            )

        pending.append(norm_tail)

    for b in range(BPC):
        # load + round x in halves so the first qkv matmul starts sooner
        # (batch 0's x was prefetched above, before the weight DMAs)
        if b == 0:
            x_s = x0_s
        else:
            x_s = sb.tile([128, N], f32, tag="x", name=f"x{b}_s")
        x_r = x_s
        if mdt != f32:
            x_r = sb.tile([128, N], mdt, tag="xr", name="x_r")
        for nf in range(2):
            sl = slice(nf * 512, (nf + 1) * 512)
            if b > 0:
                nc.sync.dma_start(out=x_s[:, sl], in_=x_ap[b, :, sl])
            if mdt != f32:
                nc.gpsimd.tensor_copy(out=x_r[:, sl], in_=x_s[:, sl])

        # --- qkv projection: psum rows m*128.. are q/k/v, each [128(h,d), N] ---
        q_p = [sb.tile([64, N], sdt, tag=f"q{i}", name=f"q{i}") for i in range(2)]
        kp_p = [sb.tile([64, N], sdt, tag=f"kp{i}", name=f"kp{i}") for i in range(2)]
        v_s = sb.tile([128, N], f32, tag="v")
        # v first: the PE transposes depend only on v, so they can fill the
        # pipeline while the q/k+r copies for the S matmuls drain
        for m in (2, 1, 0):
            ps = psmm.tile([128, N], f32, tag="mm", name=f"ps_qkv{m}")
            for nf in range(2):
                nc.tensor.matmul(
                    ps[:, nf * 512:(nf + 1) * 512],
                    lhsT=w_r[:, m * 128:(m + 1) * 128],
                    rhs=x_r[:, nf * 512:(nf + 1) * 512],
                    start=True, stop=True,
                )
            if m == 0:
                # 1/sqrt(d) score scale is folded into W_qkv's q rows on host
                for i in range(2):
                    nc.scalar.activation(
                        out=q_p[i][:], in_=ps[i * 64:(i + 1) * 64, :],
                        func=mybir.ActivationFunctionType.Copy, scale=1.0,
                    )
            elif m == 1:
                for i in range(2):
                    nc.vector.tensor_add(
                        out=kp_p[i][:], in0=ps[i * 64:(i + 1) * 64, :],
                        in1=r_s[i * 64:(i + 1) * 64, :],
                    )
            else:
                nc.vector.tensor_copy(out=v_s[:], in_=ps[:])

        # --- v^T tiles with ones column: vt1[jc][:, h, :] = [v_h^T | 1] ---
        vt1 = []
        for jc in range(8):
            ps_t = psmm.tile([128, 128], f32, tag="mm", name=f"ps_t{jc}")
            nc.tensor.transpose(ps_t[:], v_s[:, jc * 128:(jc + 1) * 128], identity[:])
            vt = vt1pool.tile([128, HEADS, D + 1], odt, tag="vt1", name=f"vt{jc}")
            nc.vector.tensor_copy(
                out=vt[:, :, D:D + 1],
                in_=ones_f[:, 0:HEADS].rearrange("p (h o) -> p h o", o=1),
            )
            nc.vector.tensor_copy(
                out=vt[:, :, 0:D],
                in_=ps_t[:].rearrange("p (h d) -> p h d", h=HEADS),
            )
            vt1.append(vt)

        out_s = sb.tile([128, N], f32, tag="out")

        # --- attention, software-pipelined across heads ---
        for h in range(4):
            lo = (h % 2) * D
            q_h = q_p[h // 2][lo:lo + D, :]
            kp_h = kp_p[h // 2][lo:lo + D, :]
            last_head = (b == BPC - 1 and h == HEADS - 1)
            own_o = []

            e_tiles = []
            if last_head:
                ps_o_pre = psacc.tile([D + 1, N], f32, tag="acc",
                                      name=f"ps_o{h}")

                def o_chunk_pre(jc, ps_o=ps_o_pre, e_tiles=e_tiles, vt1=vt1,
                                h=h, b=b, out_s=out_s):
                    for nf in range(2):
                        nc.tensor.matmul(
                            ps_o[:, nf * 512:(nf + 1) * 512],
                            lhsT=vt1[jc][:, h, :],
                            rhs=e_tiles[jc][:, nf * 512:(nf + 1) * 512],
                            start=(jc == 0), stop=(jc == 7),
                        )
                    if jc == 7:
                        finish_head(h, b, ps_o, out_s)

                own_o = [lambda jc=jc: o_chunk_pre(jc) for jc in range(8)]
            for jc in range(8):
                ps_s = psmm.tile([128, N], f32, tag="mm", name=f"ps_s{h}_{jc}")
                if S_DTYPE == "bf16":
                    nc.tensor.matmul(
                        ps_s[:, :], lhsT=kp_h[:, jc * 128:(jc + 1) * 128],
                        rhs=q_h[:, :], start=True, stop=True,
                    )
                else:
                    for nf in range(2):
                        nc.tensor.matmul(
                            ps_s[:, nf * 512:(nf + 1) * 512],
                            lhsT=kp_h[:, jc * 128:(jc + 1) * 128],
                            rhs=q_h[:, nf * 512:(nf + 1) * 512],
                            start=True, stop=True,
                        )
                e_t = epool.tile([128, N], odt, tag="e", name=f"e{h}_{jc}")
                nc.scalar.activation(
                    out=e_t[:], in_=ps_s[:], func=mybir.ActivationFunctionType.Exp
                )
                e_tiles.append(e_t)
                # splice in one of the previous head's O chunks per S chunk
                # (8 S chunks, 8 O chunks -> evenly interleaved)
                if prev_o:
                    prev_o.pop(0)()
                # the very last head has no following S phase to hide its O
                # matmuls in, so run them inline right behind each exp
                if last_head and own_o:
                    own_o.pop(0)()
            while own_o:
                own_o.pop(0)()
            emit_pending()

            if last_head:
                continue
            # this head's O matmuls, deferred into the next head's S phase:
            # O = [v^T|1]^T E accumulated over j-chunks -> [33, N]; row 32 = Z
            ps_o = psacc.tile([D + 1, N], f32, tag="acc", name=f"ps_o{h}")

            def o_chunk(jc, ps_o=ps_o, e_tiles=e_tiles, vt1=vt1, h=h, b=b,
                        out_s=out_s):
                if O_DTYPE == "bf16":
                    nc.tensor.matmul(
                        ps_o[:, :], lhsT=vt1[jc][:, h, :],
                        rhs=e_tiles[jc][:, :],
                        start=(jc == 0), stop=(jc == 7),
                    )
                else:
                    for nf in range(2):
                        nc.tensor.matmul(
                            ps_o[:, nf * 512:(nf + 1) * 512],
                            lhsT=vt1[jc][:, h, :],
                            rhs=e_tiles[jc][:, nf * 512:(nf + 1) * 512],
                            start=(jc == 0), stop=(jc == 7),
                        )
                if jc == 7:
                    finish_head(h, b, ps_o, out_s)

            prev_o.extend([lambda jc=jc: o_chunk(jc) for jc in range(8)])

    # drain: last head's O matmuls and remaining normalizes
    while prev_o:
        prev_o.pop(0)()
    emit_pending()

    for p in (dscratch, psacc, psmm, vt1pool, epool, sb, const):
        p.release()


def build_nc():
    """Build the Bass module (shared by kernel() and test harnesses)."""
    import concourse.bacc as bacc
    import concourse.tile as tile
    from concourse import mybir

    f32 = mybir.dt.float32
    nc = bacc.Bacc("TRN2", target_bir_lowering=False, debug=False,
                   num_devices=NCORES)
    x_ap = nc.dram_tensor("x", [BPC, C, N], f32, kind="ExternalInput").ap()
    wT_ap = nc.dram_tensor("wT", [C, 3 * C], f32, kind="ExternalInput").ap()
    rw_ap = nc.dram_tensor("rw2", [HEADS * D, W], f32, kind="ExternalInput").ap()
    rh_ap = nc.dram_tensor("rh2", [HEADS * D, H], f32, kind="ExternalInput").ap()
    out_ap = nc.dram_tensor("out", [BPC, C, N], f32, kind="ExternalOutput").ap()

    with tile.TileContext(nc) as tc:
        _build_kernel(nc, tc, tile, mybir, x_ap, wT_ap, rw_ap, rh_ap, out_ap)
    nc.compile()
    return nc


def make_in_maps(x, W_qkv, rw, rh):
    x_ = np.ascontiguousarray(np.asarray(x, np.float32).reshape(B, C, N))
    wT = np.ascontiguousarray(np.asarray(W_qkv, np.float32).T)
    wT[:, 0:C] *= SCALE    # fold the attention score scale into q projection
    rw_ = np.ascontiguousarray(np.asarray(rw, np.float32).reshape(HEADS * D, W))
    rh_ = np.ascontiguousarray(np.asarray(rh, np.float32).reshape(HEADS * D, H))
    return [
        {"x": x_[i * BPC:(i + 1) * BPC], "wT": wT, "rw2": rw_, "rh2": rh_}
        for i in range(NCORES)
    ]


def kernel(x, W_qkv, rw, rh):
    from concourse.bass_utils import run_bass_kernel_spmd

    nc = build_nc()
    in_maps = make_in_maps(x, W_qkv, rw, rh)
    res = None
    for attempt in range(3):
        try:
            res = run_bass_kernel_spmd(nc, in_maps, list(range(NCORES)))
            break
        except Exception:
            # transient device errors (e.g. NRT_EXEC_UNIT_UNRECOVERABLE after
            # an earlier crashed run) usually clear on retry
            if attempt == 2:
                raise
    out = np.concatenate([r["out"] for r in res.results], axis=0)
    return out.reshape(B, C, H, W).astype(np.float32)

